# revision 21
# baseline (speedup 1.0000x reference)
"""Trainium2 Bass kernel: exact 3x3x3 median blur (median of 27) over
(2,1,128,128,128), zero-padded borders, distributed over 8 NeuronCores.

Strategy:
  - fp16 end-to-end. float32->float16 rounding is monotone, and the median
    is an order statistic, so median(fp16(x)) == fp16(median(x)) exactly:
    worst-case rel err 2^-11 vs the f32 reference (gate is 2e-2), and
    exact zeros at the borders stay exact. Halves tunnel traffic and
    on-chip bandwidth vs f32.
  - cores shard (batch, W-slab): core c -> batch c//4, W rows [32*(c%4) .. +32)
    with 1-voxel halo, zero-padded host-side -> per-core input [128 H, 34 W, 130 D].
  - partitions = H (128). All compute ops are free-dim ops; the H-axis (+-1)
    combination uses partition-shifted SBUF->SBUF DMA copies of the 9 sorted
    column planes.
  - exact selection network (min/max only): z-sort3 along D (6 ops), sort9
    along W via Batcher merges (36 ops), pair-merge of (h-1,h) columns
    truncated to ranks 5..14 (48 ops), final merge rank-10-of-19 (18 ops).
    Bitwise-exact vs sort-then-index-13 (in fp16).
  - execute path: the axon redirect of run_bass_kernel_spmd rebuilds
    jit(shard_map(...)) and ships ~8MB of donated zero output buffers from
    host on every call. We run the same _bass_exec_p machinery but cache
    the jitted executable across calls and materialize the donated zero
    buffers on-device with a tiny jitted zeros producer, so the per-call
    tunnel traffic is just input (9MB) + output (8.4MB) fp16.
"""
import numpy as np

N_WIDE = 5
N_NARROW = 14
MED_REG = 'R6'
SCHED = [('op', 'z', 'MIN', 'W0', 'a', 'b'), ('op', 'z', 'MAX', 'W1', 'a', 'b'), ('op', 'z', 'MIN', 'W2', 'W0', 'c'), ('op', 'z', 'MIN', 'W3', 'W1', 'c'), ('op', 'z', 'MAX', 'W4', 'W0', 'W3'), ('op', 'z', 'MAX', 'W3', 'W1', 'c'), ('op', 'y', 'MIN', 'R0', ('W2', 0), ('W2', 1)), ('op', 'y', 'MAX', 'R1', ('W2', 0), ('W2', 1)), ('op', 'y', 'MIN', 'R2', ('W3', 0), ('W3', 1)), ('op', 'y', 'MAX', 'R3', ('W3', 0), ('W3', 1)), ('op', 'y', 'MIN', 'R4', 'R2', 'R1'), ('op', 'y', 'MAX', 'R5', 'R2', 'R1'), ('op', 'y', 'MIN', 'R1', ('W4', 0), ('W4', 1)), ('op', 'y', 'MAX', 'R2', ('W4', 0), ('W4', 1)), ('op', 'y', 'MIN', 'R6', 'R1', 'R4'), ('op', 'y', 'MAX', 'R7', 'R1', 'R4'), ('op', 'y', 'MIN', 'R1', 'R2', 'R5'), ('op', 'y', 'MAX', 'R4', 'R2', 'R5'), ('op', 'y', 'MIN', 'c0', 'R0', ('W2', 2)), ('op', 'y', 'MAX', 'R5', 'R0', ('W2', 2)), ('op', 'y', 'MIN', 'R2', 'R4', 'R5'), ('op', 'y', 'MAX', 'R0', 'R4', 'R5'), ('op', 'y', 'MIN', 'R5', 'R7', ('W3', 2)), ('op', 'y', 'MAX', 'R4', 'R7', ('W3', 2)), ('op', 'y', 'MIN', 'R7', 'R5', 'R2'), ('op', 'y', 'MAX', 'R8', 'R5', 'R2'), ('op', 'y', 'MIN', 'R5', 'R4', 'R0'), ('op', 'y', 'MAX', 'R2', 'R4', 'R0'), ('op', 'y', 'MIN', 'R4', 'R6', ('W4', 2)), ('op', 'y', 'MAX', 'R0', 'R6', ('W4', 2)), ('op', 'y', 'MIN', 'R6', 'R3', 'R0'), ('op', 'y', 'MAX', 'R9', 'R3', 'R0'), ('op', 'y', 'MIN', 'R0', 'R1', 'R6'), ('op', 'y', 'MAX', 'R3', 'R1', 'R6'), ('op', 'y', 'MIN', 'c1', 'R4', 'R7'), ('op', 'y', 'MAX', 'c2', 'R4', 'R7'), ('op', 'y', 'MIN', 'c3', 'R0', 'R8'), ('op', 'y', 'MAX', 'c4', 'R0', 'R8'), ('op', 'y', 'MIN', 'c5', 'R3', 'R5'), ('op', 'y', 'MAX', 'c6', 'R3', 'R5'), ('op', 'y', 'MIN', 'c7', 'R9', 'R2'), ('op', 'y', 'MAX', 'c8', 'R9', 'R2'), ('shiftdn_all',), ('op', 'x', 'MAX', 'R6', 'cd0', 'c0'), ('shiftup_all',), ('op', 'x', 'MAX', 'R1', 'cd1', 'c1'), ('op', 'x', 'MIN', 'R7', 'cd7', 'c7'), ('op', 'x', 'MIN', 'R4', 'cd8', 'c8'), ('op', 'x', 'MIN', 'R0', 'R4', 'R6'), ('op', 'x', 'MAX', 'R8', 'R4', 'R6'), ('op', 'x', 'MIN', 'R3', 'cd4', 'c4'), ('op', 'x', 'MAX', 'R5', 'cd4', 'c4'), ('op', 'x', 'MIN', 'R9', 'R3', 'R0'), ('op', 'x', 'MAX', 'R2', 'R3', 'R0'), ('op', 'x', 'MIN', 'R6', 'R5', 'R8'), ('op', 'x', 'MAX', 'R4', 'R5', 'R8'), ('op', 'x', 'MIN', 'R0', 'cd2', 'c2'), ('op', 'x', 'MAX', 'R3', 'R0', 'R9'), ('op', 'x', 'MAX', 'R8', 'cd2', 'c2'), ('op', 'x', 'MIN', 'R5', 'cd6', 'c6'), ('op', 'x', 'MAX', 'R0', 'cd6', 'c6'), ('op', 'x', 'MIN', 'R9', 'R0', 'R4'), ('op', 'x', 'MIN', 'R4', 'R5', 'R8'), ('op', 'x', 'MAX', 'R0', 'R5', 'R8'), ('op', 'x', 'MIN', 'R5', 'R4', 'R2'), ('op', 'x', 'MAX', 'R8', 'R4', 'R2'), ('op', 'x', 'MIN', 'R2', 'R0', 'R6'), ('op', 'x', 'MAX', 'R4', 'R0', 'R6'), ('op', 'x', 'MIN', 'R6', 'cd5', 'c5'), ('op', 'x', 'MAX', 'R0', 'cd5', 'c5'), ('op', 'x', 'MIN', 'R10', 'R6', 'R1'), ('op', 'x', 'MAX', 'R11', 'R6', 'R1'), ('op', 'x', 'MIN', 'R1', 'cd3', 'c3'), ('op', 'x', 'MAX', 'R6', 'cd3', 'c3'), ('op', 'x', 'MIN', 'R12', 'R7', 'R6'), ('op', 'x', 'MAX', 'R13', 'R7', 'R6'), ('op', 'x', 'MIN', 'R6', 'R1', 'R10'), ('op', 'x', 'MAX', 'R7', 'R1', 'R10'), ('op', 'x', 'MAX', 'R10', 'R6', 'R3'), ('op', 'w', 'MAX', 'R1', 'R10', 'cu0'), ('op', 'x', 'MIN', 'R6', 'R12', 'R11'), ('op', 'x', 'MAX', 'R3', 'R12', 'R11'), ('op', 'x', 'MIN', 'R10', 'R13', 'R0'), ('op', 'x', 'MAX', 'R12', 'R13', 'R0'), ('op', 'x', 'MIN', 'R11', 'R12', 'R9'), ('op', 'x', 'MIN', 'R0', 'R7', 'R5'), ('op', 'x', 'MAX', 'R13', 'R7', 'R5'), ('op', 'w', 'MAX', 'R9', 'R13', 'cu2'), ('op', 'w', 'MAX', 'R12', 'R0', 'cu1'), ('op', 'w', 'MIN', 'R7', 'R11', 'R12'), ('op', 'x', 'MIN', 'R5', 'R6', 'R8'), ('op', 'x', 'MAX', 'R13', 'R6', 'R8'), ('op', 'w', 'MAX', 'R0', 'R13', 'cu4'), ('op', 'w', 'MAX', 'R11', 'R5', 'cu3'), ('op', 'x', 'MIN', 'R12', 'R3', 'R2'), ('op', 'x', 'MAX', 'R6', 'R3', 'R2'), ('op', 'w', 'MIN', 'R8', 'R6', 'cu6'), ('op', 'w', 'MAX', 'R13', 'R8', 'R9'), ('op', 'w', 'MIN', 'R5', 'R12', 'cu5'), ('op', 'w', 'MAX', 'R2', 'R5', 'R7'), ('op', 'x', 'MIN', 'R3', 'R10', 'R4'), ('op', 'x', 'MAX', 'R6', 'R10', 'R4'), ('op', 'w', 'MIN', 'R9', 'R6', 'cu8'), ('op', 'w', 'MAX', 'R8', 'R9', 'R1'), ('op', 'w', 'MIN', 'R12', 'R0', 'R8'), ('op', 'w', 'MIN', 'R5', 'R13', 'R12'), ('op', 'w', 'MIN', 'R7', 'R3', 'cu7'), ('op', 'w', 'MIN', 'R10', 'R7', 'R11'), ('op', 'w', 'MAX', 'R4', 'R10', 'R2'), ('op', 'w', 'MIN', 'R6', 'R4', 'R5')]


DSEG = 32
USE_GPSIMD = False
NSEG = 128 // DSEG

_CACHE = {}


def _build_module(hin=128, hlo=0, hhi=0):
    """Module over an H-slab: hin input rows (including hlo top / hhi bottom
    halo rows provided by the host, zero-filled at volume edges); outputs
    the hin-hlo-hhi interior rows."""
    import concourse.mybir as mybir
    from concourse import bacc
    from concourse.tile import TileContext

    f16 = mybir.dt.float16
    u8 = mybir.dt.uint8
    AOT = mybir.AluOpType
    hout = hin - hlo - hhi
    nc = bacc.Bacc(None, target_bir_lowering=False)
    xin = nc.dram_tensor("x", [hin, 34, 130], f16, kind="ExternalInput")
    # argmedian index, encoded 64-l (l = i*9+j*3+k window offset); host
    # decodes via table and gathers the exact f32 value from the original.
    yout = nc.dram_tensor("y", [hout, 32, NSEG, DSEG], u8, kind="ExternalOutput")

    with TileContext(nc) as tc:
        with (
            tc.tile_pool(name="inp", bufs=1) as pin,
            tc.tile_pool(name="wide", bufs=1) as pwide,
            tc.tile_pool(name="narrow", bufs=1) as pnarrow,
            tc.tile_pool(name="colp", bufs=1) as pcol,
        ):
            in_t = pin.tile([hin, 34, 130], f16, name="in_t")
            nc.sync.dma_start(in_t[:], xin[:])
            cd_all = pin.tile([hin, 9, 32, DSEG], f16, name="cd_all")
            cu_all = pin.tile([hin, 9, 32, DSEG], f16, name="cu_all")
            nc.vector.memset(cd_all[:], 0.0)
            nc.vector.memset(cu_all[:], 0.0)
            # H+-1 shifted copies of the raw input for argmedian matching
            in_dn = pin.tile([hin, 34, 130], f16, name="in_dn")
            in_up = pin.tile([hin, 34, 130], f16, name="in_up")
            nc.vector.memset(in_dn[:], 0.0)
            nc.vector.memset(in_up[:], 0.0)
            nc.scalar.dma_start(in_dn[1:hin, :, :], in_t[0:hin - 1, :, :])
            nc.scalar.dma_start(in_up[0:hin - 1, :, :], in_t[1:hin, :, :])

            for s in range(NSEG):
                d0 = s * DSEG
                cur = {}

                c_all = pcol.tile([hin, 9, 32, DSEG], f16, name=f"c_all_{s}",
                                  tag="c_all")

                def rd(m):
                    if isinstance(m, tuple):
                        r, k = m
                        return cur[r][:, k:k + 32, :]
                    if m in ("a", "b", "c"):
                        off = {"a": 0, "b": 1, "c": 2}[m]
                        return in_t[:, :, d0 + off:d0 + off + DSEG]
                    if m.startswith("cd"):
                        return cd_all[:, int(m[2:]), :, :]
                    if m.startswith("cu"):
                        return cu_all[:, int(m[2:]), :, :]
                    if m.startswith("c"):
                        return c_all[:, int(m[1:]), :, :]
                    return cur[m][:, :, :]

                def new_tile(reg):
                    if reg.startswith("c"):
                        return rd(reg)
                    if reg.startswith("W"):
                        t = pwide.tile([hin, 34, DSEG], f16,
                                       name=f"{reg}_{s}", tag=reg)
                    else:
                        t = pnarrow.tile([hin, 32, DSEG], f16,
                                         name=f"{reg}_{s}", tag=reg)
                    cur[reg] = t
                    return t[:, :, :]

                for e in SCHED:
                    if e[0] == "op":
                        _, stage, kind, out, a, b = e
                        in0, in1 = rd(a), rd(b)
                        wide_op = out.startswith("W")
                        if not wide_op and isinstance(a, str) and a.startswith("W"):
                            in0 = cur[a][:, 0:32, :]
                        if not wide_op and isinstance(b, str) and b.startswith("W"):
                            in1 = cur[b][:, 0:32, :]
                        dst = new_tile(out)
                        op = AOT.min if kind == "MIN" else AOT.max
                        eng = nc.gpsimd if (stage == "y" and USE_GPSIMD) else nc.vector
                        eng.tensor_tensor(dst, in0, in1, op)
                    elif e[0] == "shiftdn_all":
                        nc.scalar.dma_start(cd_all[1:hin, :, :, :],
                                            c_all[0:hin - 1, :, :, :])
                    else:  # shiftup_all
                        nc.scalar.dma_start(cu_all[0:hin - 1, :, :, :],
                                            c_all[1:hin, :, :, :])

                # argmedian: for each voxel find any l with w_l == med;
                # acc = max_l is_eq(w_l, med) * (64 - l), so acc = 64 - l
                # of the first (smallest-l) match. 2 DVE ops per l.
                med = cur[MED_REG][:, :, :]
                acc = pnarrow.tile([hin, 32, DSEG], f16, name=f"acc_{s}",
                                   tag="acc")
                eqt = pnarrow.tile([hin, 32, DSEG], f16, name=f"eq_{s}",
                                   tag="eqt")
                nc.vector.memset(acc[:], 0.0)
                srcs = (in_dn, in_t, in_up)
                for l in range(27):
                    i, j, k = l // 9, (l // 3) % 3, l % 3
                    w_l = srcs[i][:, j:j + 32, d0 + k:d0 + k + DSEG]
                    nc.vector.tensor_tensor(eqt[:], w_l, med, AOT.is_equal)
                    nc.vector.scalar_tensor_tensor(
                        acc[:], eqt[:], float(64 - l), acc[:],
                        AOT.mult, AOT.max)
                idx8 = pnarrow.tile([hin, 32, DSEG], u8, name=f"idx_{s}",
                                    tag="idx8")
                nc.vector.tensor_scalar(idx8[:], acc[:], 1.0, None, AOT.mult)
                nc.sync.dma_start(yout[:, :, s, :],
                                  idx8[hlo:hlo + hout, :, :])

    nc.finalize()
    return nc


CHUNK_H = 64  # output rows per chunk in the pipelined path


def _get_module(kind="full"):
    key = "nc_" + kind
    if key not in _CACHE:
        if kind == "full":
            _CACHE[key] = _build_module(128, 0, 0)
        else:  # H-chunk of CHUNK_H output rows + 1 halo row each side
            _CACHE[key] = _build_module(CHUNK_H + 2, 1, 1)
    return _CACHE[key]


def _get_runner(kind="full"):
    """Cached fast executor: jit(shard_map(bass_exec)) built once, donated
    zero output buffers produced on-device. Mirrors the axon path of
    concourse.bass_utils.run_bass_kernel_spmd (bass2jax.run_bass_via_pjrt)
    minus its per-call rebuild + host-side zeros upload."""
    rkey = "runner_" + kind
    if rkey in _CACHE:
        return _CACHE[rkey]

    import jax
    import jax.numpy as jnp
    import concourse.mybir as mybir
    from concourse import bass2jax
    from jax.sharding import Mesh, NamedSharding, PartitionSpec
    from jax.experimental.shard_map import shard_map

    nc = _get_module(kind)
    bass2jax.install_neuronx_cc_hook()
    assert nc.dbg_addr is None
    partition_name = nc.partition_id_tensor.name if nc.partition_id_tensor else None

    in_names, out_names, out_avals = [], [], []
    for alloc in nc.m.functions[0].allocations:
        if not isinstance(alloc, mybir.MemoryLocationSet):
            continue
        name = alloc.memorylocations[0].name
        if alloc.kind == "ExternalInput":
            if name != partition_name:
                in_names.append(name)
        elif alloc.kind == "ExternalOutput":
            out_names.append(name)
            out_avals.append(jax.core.ShapedArray(
                tuple(alloc.tensor_shape), mybir.dt.np(alloc.dtype)))
    n_params, n_outs = len(in_names), len(out_avals)
    all_names = in_names + out_names
    if partition_name is not None:
        all_names = all_names + [partition_name]

    def _body(*args):
        operands = list(args)
        if partition_name is not None:
            operands.append(bass2jax.partition_id_tensor())
        outs = bass2jax._bass_exec_p.bind(
            *operands,
            out_avals=tuple(out_avals),
            in_names=tuple(all_names),
            out_names=tuple(out_names),
            lowering_input_output_aliases=(),
            sim_require_finite=True,
            sim_require_nnan=True,
            nc=nc,
        )
        return tuple(outs)

    devices = jax.devices()[:8]
    mesh = Mesh(np.asarray(devices), ("core",))
    spec = PartitionSpec("core")
    # No donate_argnums: the kernel writes every output element, so the
    # zero "output seed" operands are semantically unused — without
    # donation they survive each call and a single persistent on-device
    # dummy can be reused forever (no per-call host upload or zeros run).
    sharded = jax.jit(
        shard_map(_body, mesh=mesh, in_specs=(spec,) * (n_params + n_outs),
                  out_specs=(spec,) * n_outs, check_rep=False),
        keep_unused=True,
    )
    zshapes = [(8 * a.shape[0], *a.shape[1:]) for a in out_avals]
    zdtypes = [a.dtype for a in out_avals]
    zeros_fn = jax.jit(
        lambda: tuple(jnp.zeros(s, d) for s, d in zip(zshapes, zdtypes)),
        out_shardings=tuple(NamedSharding(mesh, spec) for _ in zshapes),
    )
    dummies = zeros_fn()
    jax.block_until_ready(dummies)
    _CACHE[rkey] = (sharded, dummies)
    return _CACHE[rkey]


def _fill_input(xh, buf):
    """xh: (2,128,128,128) fp16 (B,H,W,D) -> buf: (8*128, 34, 130) fp16 with
    zero halo borders (already zeroed once at allocation; geometry is
    static, so only the data region is rewritten per call)."""
    for core in range(8):
        b, ws = divmod(core, 4)
        dst = buf[core * 128:(core + 1) * 128]
        w0 = ws * 32 - 1
        if ws == 0:
            dst[:, 1:34, 1:129] = xh[b, :, 0:33, :]
        elif ws == 3:
            dst[:, 0:33, 1:129] = xh[b, :, w0:w0 + 33, :]
        else:
            dst[:, :, 1:129] = xh[b, :, w0:w0 + 34, :]


def _recon_tables():
    """base linear index per voxel + (64-l)->linear-delta decode table for
    gathering exact f32 medians from the 1-padded original volume."""
    if "recon" not in _CACHE:
        h = np.arange(128, dtype=np.int32)[:, None, None] * (130 * 130)
        w = np.arange(128, dtype=np.int32)[None, :, None] * 130
        d = np.arange(128, dtype=np.int32)[None, None, :]
        base = np.ascontiguousarray(h + w + d)
        delta = np.full(65, 130 * 130 + 130 + 1, dtype=np.int32)
        for l in range(27):
            i, j, k = l // 9, (l // 3) % 3, l % 3
            delta[64 - l] = i * 130 * 130 + j * 130 + k
        _CACHE["recon"] = (base, delta)
    return _CACHE["recon"]


def _fill_xpad(x):
    if "xpad" not in _CACHE:
        _CACHE["xpad"] = np.zeros((2, 130, 130, 130), np.float32)
    xpad = _CACHE["xpad"]
    xpad[:, 1:129, 1:129, 1:129] = x[:, 0]
    return xpad


def _recon_core(xpad, base, delta, idx_c, core, res):
    b, ws = divmod(core, 4)
    lin = base[:, ws * 32:ws * 32 + 32, :] + delta[idx_c]
    res[b, 0, :, ws * 32:ws * 32 + 32, :] = xpad[b].reshape(-1)[lin]


def _reconstruct(x, idx):
    """x: (2,1,128,128,128) f32 original; idx: (8,128,32,128) uint8 encoded
    argmedian -> (2,1,128,128,128) f32 exact median values."""
    base, delta = _recon_tables()
    xpad = _fill_xpad(x)
    res = np.empty((2, 1, 128, 128, 128), dtype=np.float32)
    for core in range(8):
        _recon_core(xpad, base, delta, idx[core], core, res)
    return res


def _one_pass_fast(x):
    """x: (2,1,128,128,128) f32 -> same shape median-blurred (f32)."""
    from concurrent.futures import ThreadPoolExecutor

    sharded, dummies = _get_runner("full")
    if "inbuf" not in _CACHE:
        _CACHE["inbuf"] = np.zeros((8 * 128, 34, 130), np.float16)
    if "pool" not in _CACHE:
        _CACHE["pool"] = ThreadPoolExecutor(16)
    buf = _CACHE["inbuf"]
    xh = x.astype(np.float16)[:, 0]
    _fill_input(xh, buf)
    (out,) = sharded(buf, *dummies)  # async: h2d streams in background
    base, delta = _recon_tables()
    xpad = _fill_xpad(x)  # overlaps the device round-trip
    res = np.empty((2, 1, 128, 128, 128), dtype=np.float32)

    def fetch_and_recon(shard):
        core = shard.index[0].start // 128
        idx_c = np.asarray(shard.data).reshape(128, 32, 128)
        _recon_core(xpad, base, delta, idx_c, core, res)

    list(_CACHE["pool"].map(fetch_and_recon, out.addressable_shards))
    return res


def _fill_chunk(xh, buf, chunk):
    """xh: (2,128,128,128) fp16 -> buf: (8*(CHUNK_H+2), 34, 130) for H rows
    [chunk*CHUNK_H - 1, chunk*CHUNK_H + CHUNK_H] clipped, zero elsewhere
    (borders pre-zeroed once; geometry static)."""
    hp = CHUNK_H + 2
    h0 = chunk * CHUNK_H - 1
    r0 = 0 if h0 >= 0 else 1           # dest row where data starts
    s0 = max(h0, 0)                    # source row
    s1 = min(h0 + hp, 128)             # source end
    for core in range(8):
        b, ws = divmod(core, 4)
        dst = buf[core * hp:(core + 1) * hp]
        w0 = ws * 32 - 1
        if ws == 0:
            dst[r0:r0 + s1 - s0, 1:34, 1:129] = xh[b, s0:s1, 0:33, :]
        elif ws == 3:
            dst[r0:r0 + s1 - s0, 0:33, 1:129] = xh[b, s0:s1, w0:w0 + 33, :]
        else:
            dst[r0:r0 + s1 - s0, :, 1:129] = xh[b, s0:s1, w0:w0 + 34, :]


def _one_pass_fast2(x):
    """Pipelined 2-chunk variant: chunk1's upload overlaps chunk0's
    download on the duplex axon link."""
    from concurrent.futures import ThreadPoolExecutor

    sharded, dummies = _get_runner("chunk")
    hp = CHUNK_H + 2
    nch = 128 // CHUNK_H
    if "chunkbufs" not in _CACHE:
        _CACHE["chunkbufs"] = [np.zeros((8 * hp, 34, 130), np.float16)
                               for _ in range(nch)]
    if "pool" not in _CACHE:
        _CACHE["pool"] = ThreadPoolExecutor(16)
    bufs = _CACHE["chunkbufs"]
    xh = x.astype(np.float16)[:, 0]
    outs = []
    for chunk in range(nch):
        _fill_chunk(xh, bufs[chunk], chunk)
        (o,) = sharded(bufs[chunk], *dummies)  # async dispatch
        outs.append(o)
    base, delta = _recon_tables()
    xpad = _fill_xpad(x)  # overlaps the device round-trips
    res = np.empty((2, 1, 128, 128, 128), dtype=np.float32)

    def fetch_and_recon(task):
        chunk, shard = task
        core = shard.index[0].start // CHUNK_H
        b, ws = divmod(core, 4)
        idx_c = np.asarray(shard.data).reshape(CHUNK_H, 32, 128)
        h0 = chunk * CHUNK_H
        lin = base[h0:h0 + CHUNK_H, ws * 32:ws * 32 + 32, :] + delta[idx_c]
        res[b, 0, h0:h0 + CHUNK_H, ws * 32:ws * 32 + 32, :] = \
            xpad[b].reshape(-1)[lin]

    tasks = [(chunk, shard) for chunk, o in enumerate(outs)
             for shard in o.addressable_shards]
    list(_CACHE["pool"].map(fetch_and_recon, tasks))
    return res


def _one_pass_spmd(x):
    """Fallback: the stock run_bass_kernel_spmd path."""
    from concourse.bass_utils import run_bass_kernel_spmd

    nc = _get_module("full")
    xp = np.pad(x.astype(np.float16), ((0, 0), (0, 0), (0, 0), (1, 1), (1, 1)))
    in_maps = []
    for core in range(8):
        b, ws = divmod(core, 4)
        shard = np.ascontiguousarray(xp[b, 0, :, ws * 32:ws * 32 + 34, :])
        in_maps.append({"x": shard})
    res = run_bass_kernel_spmd(nc, in_maps, core_ids=list(range(8)))
    idx = np.stack([res.results[core]["y"].reshape(128, 32, 128)
                    for core in range(8)])
    return _reconstruct(x, idx)


def _one_pass(x):
    if _CACHE.get("fast_broken"):
        return _one_pass_spmd(x)
    try:
        return _one_pass_fast2(x)
    except Exception:
        _CACHE["fast_broken"] = True
        return _one_pass_spmd(x)


def kernel(x, numpass):
    x = np.asarray(x, dtype=np.float32)
    n = int(np.asarray(numpass))
    out = x
    for _ in range(n):
        out = _one_pass(out)
    return out
